# revision 2
# baseline (speedup 1.0000x reference)
"""CenterLoss kernel for Trainium2, 8 NeuronCores, data-parallel over the batch.

Reference computation (B=65536, D=512, C=1024):
    sums_c  = segment_sum(x, t)                 # [C, D]
    counts  = bincount(t)                       # [C]
    centers = sums / max(counts, 1)
    loss    = 0.5 * sum_i ||x_i - centers[t_i]||^2

Algebraic rewrite (exact, incl. empty classes):
    loss = 0.5 * ( sum_i ||x_i||^2  -  sum_c ||S_c||^2 / max(n_c, 1) )

Per core (8192 samples), per 256-sample supertile u:
  - fp8 one-hot of the 2x128 targets, r-major layout [p, r, C] (two
    contiguous [128, 1024] DVE writes);
  - 8 DoubleRow fp8 matmuls (K=256) accumulate segment sums for all 1024
    classes across all 8 PSUM banks;
  - sum(x^2) via one ACT Square (accum_out) per 1024-sample DMA group;
  - fp8 cast of x split across engines to keep every engine under the
    DMA pace.
One-hots stay resident (64 KB/partition); the epilogue reduces them to
exact per-class counts with 64 FD-512 ones-matmuls.  Cross-core:
ReduceScatter of [C, D+1] (S || counts), per-core class-shard partial of
the second term, then a tiny ReduceScatter of the replicated [8, 2]
scalar partials (cheaper than AllReduce).
"""

import numpy as np

from concourse import bass, bacc, tile, mybir, bass_utils

B, D, C = 65536, 512, 1024
N_CORES = 8
BL = B // N_CORES          # samples per core
P = 128                    # partitions / tile rows
NT = BL // P               # 64 sample tiles per core
G = 8                      # tiles per DMA group
NG = NT // G               # 8 groups
NU = NT // 2               # 32 supertiles
NCHUNK = C // P            # 8 class chunks

_f32 = mybir.dt.float32
_bf16 = mybir.dt.bfloat16
_f16 = mybir.dt.float16
_i32 = mybir.dt.int32
_f8 = mybir.dt.float8e4
_f8e5 = mybir.dt.float8e5

_compiled = None


def _build(repeat=1):
    nc = bacc.Bacc("TRN2", target_bir_lowering=False, debug=False,
                   num_devices=N_CORES)

    x_d = nc.dram_tensor("x", [BL, D], _f32, kind="ExternalInput")
    t_d = nc.dram_tensor("t", [BL], _i32, kind="ExternalInput")
    iota_d = nc.dram_tensor("iota", [P, C], _f16, kind="ExternalInput")
    out_d = nc.dram_tensor("out", [1, 1], _f32, kind="ExternalOutput")

    rg = [list(range(N_CORES))]

    with tile.TileContext(nc) as tc:
        with (
            tc.tile_pool(name="const", bufs=1) as cpool,
            tc.tile_pool(name="xg", bufs=3) as xgpool,
            tc.tile_pool(name="work", bufs=4) as wpool,
            tc.tile_pool(name="psum", bufs=1, space="PSUM") as ppool,
            tc.tile_pool(name="dram", bufs=1, space="DRAM") as dpool,
        ):
            # ---- constants / persistent state ----
            iota_sb = cpool.tile([P, C], _f16, tag="iota")
            nc.sync.dma_start(iota_sb[:], iota_d.ap())

            ones_f32 = cpool.tile([P, 1], _f32, tag="ones_f32")
            nc.vector.memset(ones_f32[:], 1.0)
            # [p, 2, 16] DoubleRow ones-weight (r stride 16B satisfies the
            # step%16 constraint on weight APs)
            ones_f8 = cpool.tile([P, 32], _f8, tag="ones_f8")
            nc.vector.memset(ones_f8[:], 1.0)

            # sample (g, p, j): row = g*(P*G) + p*G + j  (8 KiB/partition
            # contiguous DMA lines); tile index k = g*G + j.
            t_i32 = cpool.tile([P, NT], _i32, tag="t_i32")
            nc.sync.dma_start(
                t_i32[:].rearrange("p (g j) -> p g j", j=G),
                t_d.ap().rearrange("(g p j) -> p g j", p=P, j=G))
            t_f32 = cpool.tile([P, NT], _f32, tag="t_f32")
            nc.vector.tensor_copy(t_f32[:], t_i32[:])

            # all supertile one-hots, r-major: [p, u, r, C] fp8 (exact 0/1)
            o_all = cpool.tile([P, NU * 2 * C], _f8, tag="o_all")

            # running sum-of-squares accumulator [P, 1] f32
            sq_acc = cpool.tile([P, 1], _f32, tag="sq_acc")
            nc.vector.memset(sq_acc[:], 0.0)

            # ---- PSUM: 8 banks accumulate segment sums over all tiles ----
            psum_s = [ppool.tile([P, D], _f32, tag=f"s{c}", name=f"psum_s{c}")
                      for c in range(NCHUNK)]

            xga = x_d.ap().rearrange("(g p j) d -> g p j d", p=P, j=G)

            # ---- PE warm-up burst ----
            # HAM keeps the PE at 1.2 GHz until ~3 us of *continuous*
            # execution; a DMA-paced loop never accumulates that, so pay a
            # one-time ~7 us burst of zero-weight matmuls (adds 0 to PSUM)
            # to reach 2.4 GHz.  Once warm, the loop's sub-us idle gaps stay
            # below the ~3.4 us re-throttle window.
            zero_w = cpool.tile([P, 32], _f8, tag="zero_w")
            nc.vector.memset(zero_w[:], 0.0)
            warm_x = cpool.tile([P, 2 * D], _f8, tag="warm_x")
            nc.vector.memset(warm_x[:], 0.0)
            z3 = zero_w[:].rearrange("p (r m) -> p r m", r=2)
            wx3 = warm_x[:].rearrange("p (r d) -> p r d", r=2)
            for w in range(32):
                nc.tensor.matmul(
                    psum_s[w % NCHUNK][0:16, :], lhsT=z3, rhs=wx3,
                    perf_mode=mybir.MatmulPerfMode.DoubleRow,
                    start=(w < NCHUNK), stop=(w >= 32 - NCHUNK),
                    skip_group_check=True,
                )

            # ---- main loop ----
            def main_loop():
                for g in range(NG):
                    xg = xgpool.tile([P, G * D], _f32, tag="xg")
                    nc.sync.dma_start(
                        xg[:].rearrange("p (j d) -> p j d", j=G), xga[g])

                    # one ACT Square + accum for the whole 4096-elem group
                    sqs = wpool.tile([P, G * D], _f8e5, tag="sqs")
                    sqp = wpool.tile([P, 1], _f32, tag="sqp")
                    nc.scalar.activation(
                        sqs[:], xg[:], mybir.ActivationFunctionType.Square,
                        accum_out=sqp[:, 0:1])
                    nc.vector.tensor_tensor(
                        sq_acc[:], sq_acc[:], sqp[:], mybir.AluOpType.add)

                    for h in range(G // 2):
                        u = g * (G // 2) + h
                        xv2 = xg[:, h * 2 * D:(h + 1) * 2 * D]   # [P, 1024]

                        # fp8 cast of the supertile pair; alternate DVE/ACT
                        # (measured: DVE 379ns, ACT 750ns per [128,1024];
                        # GPSIMD is far too slow for this)
                        xf8 = wpool.tile([P, 2 * D], _f8, tag="xf8")
                        if u % 2 == 0:
                            nc.vector.tensor_copy(xf8[:], xv2)
                        else:
                            nc.scalar.copy(xf8[:], xv2)

                        # fp8 one-hots, r-major contiguous [p, r, C]
                        ob = o_all[:, u * 2 * C:(u + 1) * 2 * C]
                        for r in range(2):
                            nc.vector.tensor_scalar(
                                ob[:, r * C:(r + 1) * C], iota_sb[:],
                                t_f32[:, 2 * u + r:2 * u + r + 1], None,
                                mybir.AluOpType.is_equal,
                            )

                        # segment-sum DoubleRow matmuls (K=256 per supertile)
                        x3 = xf8[:].rearrange("p (r d) -> p r d", r=2)
                        o3 = ob.rearrange("p (r c) -> p r c", r=2)
                        for c in range(NCHUNK):
                            nc.tensor.matmul(
                                psum_s[c][:],
                                lhsT=o3[:, :, c * P:(c + 1) * P],
                                rhs=x3,
                                perf_mode=mybir.MatmulPerfMode.DoubleRow,
                                start=(u == 0), stop=(u == NU - 1),
                            )

            if repeat == 1:
                main_loop()
            else:
                with tc.For_i(0, repeat, 1):
                    main_loop()

            # ---- epilogue: flush S, counts, sumsq ----
            # bf16 cross-core payload; counts stay exact (integers < 256).
            s_sb = cpool.tile([P, NCHUNK * D], _bf16, tag="s_sb")
            # flush banks 6,7 first (DVE) so PE count matmuls can start on
            # the freed banks while ACT flushes the rest.
            nc.vector.tensor_copy(s_sb[:, 6 * D:7 * D], psum_s[6][:])
            nc.vector.tensor_copy(s_sb[:, 7 * D:8 * D], psum_s[7][:])
            for c in range(6):
                if c % 3 == 2:
                    nc.vector.tensor_copy(s_sb[:, c * D:(c + 1) * D],
                                          psum_s[c][:])
                else:
                    nc.scalar.copy(s_sb[:, c * D:(c + 1) * D], psum_s[c][:])

            # counts: 64 FD-512 ones-matmuls over the resident one-hots
            ones3 = ones_f8[:].rearrange("p (r m) -> p r m", r=2)
            cnt_ps = [ppool.tile([16, D], _f32, tag=f"s{6 + jh}",
                                 name=f"cnt_ps{jh}") for jh in range(2)]
            o4 = o_all[:].rearrange("p (u r c) -> p u r c", u=NU, r=2)
            for u in range(NU):
                for jh in range(2):
                    nc.tensor.matmul(
                        cnt_ps[jh][:],
                        lhsT=ones3,
                        rhs=o4[:, u, :, jh * D:(jh + 1) * D],
                        perf_mode=mybir.MatmulPerfMode.DoubleRow,
                        start=(u == 0), stop=(u == NU - 1),
                    )
            cnt_sb = cpool.tile([1, C], _bf16, tag="cnt_sb")
            for jh in range(2):
                nc.vector.tensor_copy(cnt_sb[:, jh * D:(jh + 1) * D],
                                      cnt_ps[jh][0:1, :])

            # sumsq partial: reduce [P, 1] across partitions
            sq_ps = ppool.tile([1, 1], _f32, tag="s0", name="sq_ps")
            nc.tensor.matmul(sq_ps[:], lhsT=ones_f32[:], rhs=sq_acc[:, 0:1],
                             start=True, stop=True)

            # ---- assemble ReduceScatter input [C, D+1] = [S | counts] ----
            rs_in = dpool.tile([C, D + 1], _bf16, tag="rs_in")
            for c in range(NCHUNK):
                nc.sync.dma_start(rs_in[c * P:(c + 1) * P, 0:D],
                                  s_sb[:, c * D:(c + 1) * D])
            nc.sync.dma_start(rs_in[0:C, D:D + 1], cnt_sb[0:1, 0:C])

            rs_out = dpool.tile([C // N_CORES, D + 1], _bf16, tag="rs_out")
            nc.gpsimd.collective_compute(
                "ReduceScatter", mybir.AluOpType.add, replica_groups=rg,
                ins=[rs_in.opt()], outs=[rs_out.opt()],
            )

            # ---- per-core class-shard term: sum_c ||S_c||^2 / max(n_c,1) ----
            sh = cpool.tile([P, D + 1], _bf16, tag="sh")
            nc.sync.dma_start(sh[:], rs_out[:])

            q = cpool.tile([P, 1], _f32, tag="q")
            qscr = wpool.tile([P, D], _f32, tag="qscr")
            nc.vector.tensor_tensor(qscr[:], sh[:, 0:D], sh[:, 0:D],
                                    mybir.AluOpType.mult)
            nc.vector.tensor_reduce(q[:, 0:1], qscr[:],
                                    axis=mybir.AxisListType.X,
                                    op=mybir.AluOpType.add)
            nmax = cpool.tile([P, 1], _f32, tag="nmax")
            nc.vector.tensor_scalar_max(nmax[:], sh[:, D:D + 1], 1.0)
            rinv = cpool.tile([P, 1], _f32, tag="rinv")
            nc.vector.reciprocal(rinv[:], nmax[:])
            bpart = cpool.tile([P, 1], _f32, tag="bpart")
            nc.vector.tensor_tensor(bpart[:], q[:], rinv[:],
                                    mybir.AluOpType.mult)
            b_ps = ppool.tile([1, 1], _f32, tag="s1", name="b_ps")
            nc.tensor.matmul(b_ps[:], lhsT=ones_f32[:], rhs=bpart[:, 0:1],
                             start=True, stop=True)

            # ---- scalar cross-core reduce: RS of replicated [8, 2] ----
            par_sb = cpool.tile([1, 2 * N_CORES], _f32, tag="par_sb")
            for m in range(N_CORES):
                nc.vector.tensor_copy(par_sb[0:1, 2 * m:2 * m + 1], sq_ps[:])
                nc.vector.tensor_copy(par_sb[0:1, 2 * m + 1:2 * m + 2],
                                      b_ps[:])
            rs2_in = dpool.tile([N_CORES, 2], _f32, tag="rs2_in")
            nc.sync.dma_start(rs2_in[:], par_sb[:])
            rs2_out = dpool.tile([1, 2], _f32, tag="rs2_out")
            nc.gpsimd.collective_compute(
                "ReduceScatter", mybir.AluOpType.add, replica_groups=rg,
                ins=[rs2_in.opt()], outs=[rs2_out.opt()],
            )
            fin = cpool.tile([1, 2], _f32, tag="fin")
            nc.sync.dma_start(fin[:], rs2_out[:])

            loss_sb = cpool.tile([1, 1], _f32, tag="loss_sb")
            nc.vector.tensor_tensor(loss_sb[:], fin[0:1, 0:1], fin[0:1, 1:2],
                                    mybir.AluOpType.subtract)
            nc.vector.tensor_scalar_mul(loss_sb[:], loss_sb[:], 0.5)
            nc.sync.dma_start(out_d.ap(), loss_sb[:])

    nc.compile()
    return nc


def _get_compiled():
    global _compiled
    if _compiled is None:
        _compiled = _build()
    return _compiled


_IOTA = np.tile(np.arange(C, dtype=np.float16), (P, 1))


def make_in_maps(inputs, targets):
    x = np.ascontiguousarray(np.asarray(inputs, dtype=np.float32))
    t = np.ascontiguousarray(np.asarray(targets).astype(np.int32))
    assert x.shape == (B, D) and t.shape == (B,)
    return [
        {
            "x": x[c * BL:(c + 1) * BL],
            "t": t[c * BL:(c + 1) * BL],
            "iota": _IOTA,
        }
        for c in range(N_CORES)
    ]


def kernel(inputs, targets, num_classes=C, **_ignored):
    assert int(num_classes) == C
    nc = _get_compiled()
    res = bass_utils.run_bass_kernel_spmd(
        nc, make_in_maps(inputs, targets), core_ids=list(range(N_CORES)))
    return np.asarray(res.results[0]["out"], dtype=np.float32).reshape(())


# revision 3
# speedup vs baseline: 1.1525x; 1.1525x over previous
"""CenterLoss kernel for Trainium2, 8 NeuronCores, data-parallel over the batch.

Reference computation (B=65536, D=512, C=1024):
    sums_c  = segment_sum(x, t)                 # [C, D]
    counts  = bincount(t)                       # [C]
    centers = sums / max(counts, 1)
    loss    = 0.5 * sum_i ||x_i - centers[t_i]||^2

Algebraic rewrite (exact, incl. empty classes):
    loss = 0.5 * ( sum_i ||x_i||^2  -  sum_c ||S_c||^2 / max(n_c, 1) )

Per core (8192 samples), per 256-sample supertile u:
  - fp8 one-hot of the 2x128 targets, r-major layout [p, r, C] (two
    contiguous [128, 1024] DVE writes);
  - 8 DoubleRow fp8 matmuls (K=256) accumulate segment sums for all 1024
    classes across all 8 PSUM banks;
  - sum(x^2) via one ACT Square (accum_out) per 1024-sample DMA group;
  - fp8 cast of x split across engines to keep every engine under the
    DMA pace.
One-hots stay resident (64 KB/partition); the epilogue reduces them to
exact per-class counts with 64 FD-512 ones-matmuls.  Cross-core:
ReduceScatter of [C, D+1] (S || counts), per-core class-shard partial of
the second term, then a tiny ReduceScatter of the replicated [8, 2]
scalar partials (cheaper than AllReduce).
"""

import numpy as np

from concourse import bass, bacc, tile, mybir, bass_utils

B, D, C = 65536, 512, 1024
N_CORES = 8
BL = B // N_CORES          # samples per core
P = 128                    # partitions / tile rows
NT = BL // P               # 64 sample tiles per core
G = 8                      # tiles per DMA group
NG = NT // G               # 8 groups
NU = NT // 2               # 32 supertiles
NCHUNK = C // P            # 8 class chunks

_f32 = mybir.dt.float32
_bf16 = mybir.dt.bfloat16
_f16 = mybir.dt.float16
_i32 = mybir.dt.int32
_f8 = mybir.dt.float8e4
_f8e5 = mybir.dt.float8e5

_compiled = None


def _build(repeat=1):
    nc = bacc.Bacc("TRN2", target_bir_lowering=False, debug=False,
                   num_devices=N_CORES)

    x_d = nc.dram_tensor("x", [BL, D], _f32, kind="ExternalInput")
    t_d = nc.dram_tensor("t", [BL], _i32, kind="ExternalInput")
    iota_d = nc.dram_tensor("iota", [P, C], _f16, kind="ExternalInput")
    out_d = nc.dram_tensor("out", [1, 1], _f32, kind="ExternalOutput")

    rg = [list(range(N_CORES))]

    with tile.TileContext(nc) as tc:
        with (
            tc.tile_pool(name="const", bufs=1) as cpool,
            tc.tile_pool(name="xg", bufs=4) as xgpool,
            tc.tile_pool(name="work", bufs=6) as wpool,
            tc.tile_pool(name="psum", bufs=1, space="PSUM") as ppool,
            tc.tile_pool(name="dram", bufs=1, space="DRAM") as dpool,
        ):
            # ---- constants / persistent state ----
            iota_sb = cpool.tile([P, C], _f16, tag="iota")
            nc.sync.dma_start(iota_sb[:], iota_d.ap())

            ones_f32 = cpool.tile([P, 1], _f32, tag="ones_f32")
            nc.vector.memset(ones_f32[:], 1.0)
            # [p, 2, 16] DoubleRow ones-weight (r stride 16B satisfies the
            # step%16 constraint on weight APs)
            ones_f8 = cpool.tile([P, 32], _f8, tag="ones_f8")
            nc.vector.memset(ones_f8[:], 1.0)

            # sample (g, p, j): row = g*(P*G) + p*G + j  (8 KiB/partition
            # contiguous DMA lines); tile index k = g*G + j.
            t_i32 = cpool.tile([P, NT], _i32, tag="t_i32")
            nc.sync.dma_start(
                t_i32[:].rearrange("p (g j) -> p g j", j=G),
                t_d.ap().rearrange("(g p j) -> p g j", p=P, j=G))
            t_f32 = cpool.tile([P, NT], _f32, tag="t_f32")
            nc.vector.tensor_copy(t_f32[:], t_i32[:])

            # all supertile one-hots, r-major: [p, u, r, C] fp8 (exact 0/1)
            o_all = cpool.tile([P, NU * 2 * C], _f8, tag="o_all")

            # running sum-of-squares accumulator [P, 1] f32
            sq_acc = cpool.tile([P, 1], _f32, tag="sq_acc")
            nc.vector.memset(sq_acc[:], 0.0)

            # ---- PSUM: 8 banks accumulate segment sums over all tiles ----
            psum_s = [ppool.tile([P, D], _f32, tag=f"s{c}", name=f"psum_s{c}")
                      for c in range(NCHUNK)]

            xga = x_d.ap().rearrange("(g p j) d -> g p j d", p=P, j=G)

            # ---- PE warm-up burst ----
            # HAM keeps the PE at 1.2 GHz until ~3 us of *continuous*
            # execution; a DMA-paced loop never accumulates that, so pay a
            # one-time ~7 us burst of zero-weight matmuls (adds 0 to PSUM)
            # to reach 2.4 GHz.  Once warm, the loop's sub-us idle gaps stay
            # below the ~3.4 us re-throttle window.
            zero_w = cpool.tile([P, 32], _f8, tag="zero_w")
            nc.vector.memset(zero_w[:], 0.0)
            warm_x = cpool.tile([P, 2 * D], _f8, tag="warm_x")
            nc.vector.memset(warm_x[:], 0.0)
            z3 = zero_w[:].rearrange("p (r m) -> p r m", r=2)
            wx3 = warm_x[:].rearrange("p (r d) -> p r d", r=2)
            for w in range(32):
                nc.tensor.matmul(
                    psum_s[w % NCHUNK][0:16, :], lhsT=z3, rhs=wx3,
                    perf_mode=mybir.MatmulPerfMode.DoubleRow,
                    start=(w < NCHUNK), stop=(w >= 32 - NCHUNK),
                    skip_group_check=True,
                )

            # ---- main loop ----
            def main_loop():
                for g in range(NG):
                    xg = xgpool.tile([P, G * D], _f32, tag="xg")
                    nc.sync.dma_start(
                        xg[:].rearrange("p (j d) -> p j d", j=G), xga[g])

                    # one ACT Square + accum for the whole 4096-elem group
                    sqs = wpool.tile([P, G * D], _f8e5, tag="sqs")
                    sqp = wpool.tile([P, 1], _f32, tag="sqp")
                    nc.scalar.activation(
                        sqs[:], xg[:], mybir.ActivationFunctionType.Square,
                        accum_out=sqp[:, 0:1])
                    nc.vector.tensor_tensor(
                        sq_acc[:], sq_acc[:], sqp[:], mybir.AluOpType.add)

                    for h in range(G // 2):
                        u = g * (G // 2) + h
                        xv2 = xg[:, h * 2 * D:(h + 1) * 2 * D]   # [P, 1024]

                        # fp8 cast of the supertile pair; alternate DVE/ACT
                        # (measured: DVE 379ns, ACT 750ns per [128,1024];
                        # GPSIMD is far too slow for this)
                        xf8 = wpool.tile([P, 2 * D], _f8, tag="xf8")
                        if u % 2 == 0:
                            nc.vector.tensor_copy(xf8[:], xv2)
                        else:
                            nc.scalar.copy(xf8[:], xv2)

                        # fp8 one-hots, r-major contiguous [p, r, C]
                        ob = o_all[:, u * 2 * C:(u + 1) * 2 * C]
                        for r in range(2):
                            nc.vector.tensor_scalar(
                                ob[:, r * C:(r + 1) * C], iota_sb[:],
                                t_f32[:, 2 * u + r:2 * u + r + 1], None,
                                mybir.AluOpType.is_equal,
                            )

                        # segment-sum DoubleRow matmuls (K=256 per supertile)
                        x3 = xf8[:].rearrange("p (r d) -> p r d", r=2)
                        o3 = ob.rearrange("p (r c) -> p r c", r=2)
                        for c in range(NCHUNK):
                            nc.tensor.matmul(
                                psum_s[c][:],
                                lhsT=o3[:, :, c * P:(c + 1) * P],
                                rhs=x3,
                                perf_mode=mybir.MatmulPerfMode.DoubleRow,
                                start=(u == 0), stop=(u == NU - 1),
                            )

            if repeat == 1:
                main_loop()
            else:
                with tc.For_i(0, repeat, 1):
                    main_loop()

            # ---- epilogue: flush S, counts, sumsq ----
            # bf16 cross-core payload; counts stay exact (integers < 256).
            s_sb = cpool.tile([P, NCHUNK * D], _bf16, tag="s_sb")
            # flush banks 6,7 first (DVE) so PE count matmuls can start on
            # the freed banks while ACT flushes the rest.
            nc.vector.tensor_copy(s_sb[:, 6 * D:7 * D], psum_s[6][:])
            nc.vector.tensor_copy(s_sb[:, 7 * D:8 * D], psum_s[7][:])
            for c in range(6):
                if c % 3 == 2:
                    nc.vector.tensor_copy(s_sb[:, c * D:(c + 1) * D],
                                          psum_s[c][:])
                else:
                    nc.scalar.copy(s_sb[:, c * D:(c + 1) * D], psum_s[c][:])

            # counts: 64 FD-512 ones-matmuls over the resident one-hots
            ones3 = ones_f8[:].rearrange("p (r m) -> p r m", r=2)
            cnt_ps = [ppool.tile([16, D], _f32, tag=f"s{6 + jh}",
                                 name=f"cnt_ps{jh}") for jh in range(2)]
            o4 = o_all[:].rearrange("p (u r c) -> p u r c", u=NU, r=2)
            for u in range(NU):
                for jh in range(2):
                    nc.tensor.matmul(
                        cnt_ps[jh][:],
                        lhsT=ones3,
                        rhs=o4[:, u, :, jh * D:(jh + 1) * D],
                        perf_mode=mybir.MatmulPerfMode.DoubleRow,
                        start=(u == 0), stop=(u == NU - 1),
                    )
            cnt_sb = cpool.tile([1, C], _bf16, tag="cnt_sb")
            for jh in range(2):
                nc.vector.tensor_copy(cnt_sb[:, jh * D:(jh + 1) * D],
                                      cnt_ps[jh][0:1, :])

            # sumsq partial: reduce [P, 1] across partitions
            sq_ps = ppool.tile([1, 1], _f32, tag="s0", name="sq_ps")
            nc.tensor.matmul(sq_ps[:], lhsT=ones_f32[:], rhs=sq_acc[:, 0:1],
                             start=True, stop=True)

            # ---- assemble ReduceScatter input [C, D+1] = [S | counts] ----
            rs_in = dpool.tile([C, D + 1], _bf16, tag="rs_in")
            for c in range(NCHUNK):
                nc.sync.dma_start(rs_in[c * P:(c + 1) * P, 0:D],
                                  s_sb[:, c * D:(c + 1) * D])
            nc.sync.dma_start(rs_in[0:C, D:D + 1], cnt_sb[0:1, 0:C])

            rs_out = dpool.tile([C // N_CORES, D + 1], _bf16, tag="rs_out")
            nc.gpsimd.collective_compute(
                "ReduceScatter", mybir.AluOpType.add, replica_groups=rg,
                ins=[rs_in.opt()], outs=[rs_out.opt()],
            )

            # ---- per-core class-shard term: sum_c ||S_c||^2 / max(n_c,1) ----
            sh = cpool.tile([P, D + 1], _bf16, tag="sh")
            nc.sync.dma_start(sh[:], rs_out[:])

            q = cpool.tile([P, 1], _f32, tag="q")
            qscr = wpool.tile([P, D], _f32, tag="qscr")
            nc.vector.tensor_tensor(qscr[:], sh[:, 0:D], sh[:, 0:D],
                                    mybir.AluOpType.mult)
            nc.vector.tensor_reduce(q[:, 0:1], qscr[:],
                                    axis=mybir.AxisListType.X,
                                    op=mybir.AluOpType.add)
            nmax = cpool.tile([P, 1], _f32, tag="nmax")
            nc.vector.tensor_scalar_max(nmax[:], sh[:, D:D + 1], 1.0)
            rinv = cpool.tile([P, 1], _f32, tag="rinv")
            nc.vector.reciprocal(rinv[:], nmax[:])
            bpart = cpool.tile([P, 1], _f32, tag="bpart")
            nc.vector.tensor_tensor(bpart[:], q[:], rinv[:],
                                    mybir.AluOpType.mult)
            b_ps = ppool.tile([1, 1], _f32, tag="s1", name="b_ps")
            nc.tensor.matmul(b_ps[:], lhsT=ones_f32[:], rhs=bpart[:, 0:1],
                             start=True, stop=True)

            # ---- scalar cross-core reduce: RS of replicated [8, 2] ----
            par_sb = cpool.tile([1, 2 * N_CORES], _f32, tag="par_sb")
            for m in range(N_CORES):
                nc.vector.tensor_copy(par_sb[0:1, 2 * m:2 * m + 1], sq_ps[:])
                nc.vector.tensor_copy(par_sb[0:1, 2 * m + 1:2 * m + 2],
                                      b_ps[:])
            rs2_in = dpool.tile([N_CORES, 2], _f32, tag="rs2_in")
            nc.sync.dma_start(rs2_in[:], par_sb[:])
            rs2_out = dpool.tile([1, 2], _f32, tag="rs2_out")
            nc.gpsimd.collective_compute(
                "ReduceScatter", mybir.AluOpType.add, replica_groups=rg,
                ins=[rs2_in.opt()], outs=[rs2_out.opt()],
            )
            fin = cpool.tile([1, 2], _f32, tag="fin")
            nc.sync.dma_start(fin[:], rs2_out[:])

            loss_sb = cpool.tile([1, 1], _f32, tag="loss_sb")
            nc.vector.tensor_tensor(loss_sb[:], fin[0:1, 0:1], fin[0:1, 1:2],
                                    mybir.AluOpType.subtract)
            nc.vector.tensor_scalar_mul(loss_sb[:], loss_sb[:], 0.5)
            nc.sync.dma_start(out_d.ap(), loss_sb[:])

    nc.compile()
    return nc


def _get_compiled():
    global _compiled
    if _compiled is None:
        _compiled = _build()
    return _compiled


_IOTA = np.tile(np.arange(C, dtype=np.float16), (P, 1))


def make_in_maps(inputs, targets):
    x = np.ascontiguousarray(np.asarray(inputs, dtype=np.float32))
    t = np.ascontiguousarray(np.asarray(targets).astype(np.int32))
    assert x.shape == (B, D) and t.shape == (B,)
    return [
        {
            "x": x[c * BL:(c + 1) * BL],
            "t": t[c * BL:(c + 1) * BL],
            "iota": _IOTA,
        }
        for c in range(N_CORES)
    ]


def kernel(inputs, targets, num_classes=C, **_ignored):
    assert int(num_classes) == C
    nc = _get_compiled()
    res = bass_utils.run_bass_kernel_spmd(
        nc, make_in_maps(inputs, targets), core_ids=list(range(N_CORES)))
    return np.asarray(res.results[0]["out"], dtype=np.float32).reshape(())
